# revision 1
# baseline (speedup 1.0000x reference)
"""BasicTransformer Trainium2 kernel (Bass/Tile), data-parallel over batch on 8 cores.

Per batch b (one NeuronCore each):
    e   = embed[x[b]]                    (T, D)   indirect-DMA gather
    q/k/v = W @ e^T                      PE, fp32r
    s   = (q^T k) * 1/sqrt(D)            PE -> PSUM per 128-query chunk
    p   = exp(s - rowmax)                DVE rowmax + ACT exp (accum -> l)
    y   = (p @ v) / l                    PE on p^T (PE-transposed), DVE scale
    z   = relu(lin_w @ y + lin_b)        PE + ACT (accum -> sum over t)
    out = sigmoid(clf_w . mean + clf_b)  PE + ACT

t-order inside the kernel is a fixed permutation of the true t-order; the
computation is permutation-invariant over t (softmax over keys, p@v
contraction, mean over t), so the final (1,) output is unaffected.
"""

import math
import os

import numpy as np

B, T, D, VOCAB = 8, 2048, 512, 32000
P = 128
TC = T // P          # 16 t-chunks
DC = D // P          # 4 d-chunks
NB = T // 512        # 4 blocks of 512 along t
SCALE = 1.0 / math.sqrt(D)
N_CORES = 8

_COMPILED = {}


def _build(iters=1, mm_dtype="f32r"):
    import concourse.bacc as bacc
    import concourse.mybir as mybir
    import concourse.tile as tile
    from concourse.masks import make_identity

    dt = mybir.dt

    mdt = dt.float32r if mm_dtype == "f32r" else dt.float32

    nc = bacc.Bacc("TRN2", target_bir_lowering=False, debug=False)

    x_d = nc.declare_dram_parameter("x", [T], dt.int32, isOutput=False)
    emb_d = nc.declare_dram_parameter("embed", [VOCAB + 1, D], dt.float32, isOutput=False)
    wq_d = nc.declare_dram_parameter("W_q", [D, D], dt.float32, isOutput=False)
    wk_d = nc.declare_dram_parameter("W_k", [D, D], dt.float32, isOutput=False)
    wv_d = nc.declare_dram_parameter("W_v", [D, D], dt.float32, isOutput=False)
    lw_d = nc.declare_dram_parameter("lin_w", [D, D], dt.float32, isOutput=False)
    lb_d = nc.declare_dram_parameter("lin_b", [D], dt.float32, isOutput=False)
    cw_d = nc.declare_dram_parameter("clf_w", [D], dt.float32, isOutput=False)
    cb_d = nc.declare_dram_parameter("clf_b", [1], dt.float32, isOutput=False)
    out_d = nc.declare_dram_parameter("out", [iters, 1], dt.float32, isOutput=True)

    with tile.TileContext(nc) as tc:
        with tc.tile_pool(name="const", bufs=1) as cpool:
            ident = cpool.tile([P, P], dt.float32, tag="ident", name="ident")
            make_identity(nc, ident[:])
            identr = cpool.tile([P, P], dt.float32r, tag="identr", name="identr")
            nc.vector.tensor_copy(identr[:], ident[:])

            for it in range(iters):
                _body(nc, tc, mybir, dt, mdt, (ident, ident if mdt == dt.float32 else identr),
                      x_d, emb_d, wq_d, wk_d, wv_d, lw_d, lb_d, cw_d, cb_d,
                      out_d.ap()[it:it + 1, :])

    nc.compile()
    return nc


def _body(nc, tc, mybir, dt, mdt, idents,
          x_d, emb_d, wq_d, wk_d, wv_d, lw_d, lb_d, cw_d, cb_d, out_ap):
    import concourse.bass as bass

    ident, identm = idents

    AF = mybir.ActivationFunctionType
    AX = mybir.AxisListType
    ALU = mybir.AluOpType

    # alternate DVE / ACT for PSUM->SBUF copies to balance engine load
    _cp = [0]

    def copy_ps(out, in_):
        if _cp[0] % 2 == 0:
            nc.vector.tensor_copy(out, in_)
        else:
            nc.scalar.copy(out, in_)
        _cp[0] += 1

    with tc.tile_pool(name="persist", bufs=1) as pp:
        Q_dt = [pp.tile([P, T], mdt, tag=f"q{d}", name=f"q{d}") for d in range(DC)]
        K_dt = [pp.tile([P, T], mdt, tag=f"k{d}", name=f"k{d}") for d in range(DC)]
        V_td = [pp.tile([P, D], mdt, tag=f"v{c}", name=f"v{c}") for c in range(TC)]
        LwT = [pp.tile([P, D], mdt, tag=f"lwt{f}", name=f"lwt{f}") for f in range(DC)]
        linb = pp.tile([P, DC], dt.float32, tag="linb", name="linb")
        clfw = pp.tile([P, DC], dt.float32, tag="clfw", name="clfw")
        clfb = pp.tile([1, 1], dt.float32, tag="clfb", name="clfb")
        zsum = [pp.tile([P, NB], dt.float32, tag=f"zs{d}", name=f"zs{d}") for d in range(DC)]

        nc.sync.dma_start(out=linb[:], in_=lb_d.ap().rearrange("(c p) -> p c", p=P))
        nc.sync.dma_start(out=clfw[:], in_=cw_d.ap().rearrange("(c p) -> p c", p=P))
        nc.sync.dma_start(out=clfb[:], in_=cb_d.ap().unsqueeze(1))

        # ---------------- setup: gather, transposes, QKV projections ------
        with tc.tile_pool(name="setup", bufs=1) as sp, \
             tc.tile_pool(name="etf_pool", bufs=5) as efp, \
             tc.tile_pool(name="wtmp", bufs=2) as wp, \
             tc.tile_pool(name="setup_ps", bufs=4, space="PSUM") as spp, \
             tc.tile_pool(name="qkv_ps", bufs=2, space="PSUM") as qpp:

            idx = sp.tile([P, TC], dt.int32, tag="idx", name="idx")
            nc.sync.dma_start(out=idx[:], in_=x_d.ap().rearrange("(p c) -> p c", c=TC))

            E_ft = [sp.tile([P, T], mdt, tag=f"eft{f}", name=f"eft{f}") for f in range(DC)]

            def gather_group(g):
                etf = [efp.tile([P, D], mdt, tag="etf", name="etf") for _ in range(4)]
                for s in range(4):
                    c = g * 4 + s
                    nc.gpsimd.indirect_dma_start(
                        out=etf[s][:],
                        out_offset=None,
                        in_=emb_d.ap(),
                        in_offset=bass.IndirectOffsetOnAxis(ap=idx[:, c:c + 1], axis=0),
                    )
                for f in range(DC):
                    tp = spp.tile([P, 4, P], mdt, tag="etp", name="etp")
                    for s in range(4):
                        nc.tensor.transpose(tp[:, s, :], etf[s][:, f * P:(f + 1) * P],
                                            identm[:])
                    copy_ps(E_ft[f][:, g * 512:(g + 1) * 512], tp[:])

            def transpose_w(w_dram, dst):
                """dst[f][p_f, d] = W[d, f]  (4 tiles [128, 512])"""
                wsb = [wp.tile([P, D], dt.float32, tag=f"wL{d2}", name=f"wL{d2}", bufs=2) for d2 in range(DC)]
                for d2 in range(DC):
                    nc.sync.dma_start(out=wsb[d2][:],
                                      in_=w_dram.ap()[d2 * P:(d2 + 1) * P, :])
                for f in range(DC):
                    tp = spp.tile([P, 4, P], dt.float32, tag="etp", name="wtp")
                    for d2 in range(DC):
                        nc.tensor.transpose(tp[:, d2, :], wsb[d2][:, f * P:(f + 1) * P],
                                            ident[:])
                    copy_ps(dst[f][:], tp[:])

            # W^T for all three projection matrices up front (independent of
            # the gathers, so the PE has work from t=0), then per-gather-group
            # interleaved Q/K/V so the last gather only gates ~24 matmuls.
            WqT = [wp.tile([P, D], mdt, tag=f"wqT{f}", name=f"wqT{f}", bufs=1) for f in range(DC)]
            WkT = [wp.tile([P, D], mdt, tag=f"wkT{f}", name=f"wkT{f}", bufs=1) for f in range(DC)]
            WvT = [wp.tile([P, D], mdt, tag=f"wvT{f}", name=f"wvT{f}", bufs=1) for f in range(DC)]
            transpose_w(wq_d, WqT)
            transpose_w(wk_d, WkT)
            transpose_w(wv_d, WvT)
            transpose_w(lw_d, LwT)
            for g in range(TC // 4):
                gather_group(g)
                tb = g
                for wT, out_tiles in ((WqT, Q_dt), (WkT, K_dt)):
                    for dd in range(DC):
                        ps = qpp.tile([P, 512], dt.float32, tag="qkv", name="qkv")
                        for f in range(DC):
                            nc.tensor.matmul(ps[:], wT[f][:, dd * P:(dd + 1) * P],
                                             E_ft[f][:, tb * 512:(tb + 1) * 512],
                                             start=(f == 0), stop=(f == DC - 1))
                        copy_ps(out_tiles[dd][:, tb * 512:(tb + 1) * 512], ps[:])
                for c in range(g * 4, g * 4 + 4):
                    ps = qpp.tile([P, 512], dt.float32, tag="qkv", name="qkv")
                    for f in range(DC):
                        nc.tensor.matmul(ps[:], E_ft[f][:, c * P:(c + 1) * P],
                                         WvT[f][:],
                                         start=(f == 0), stop=(f == DC - 1))
                    copy_ps(V_td[c][:], ps[:])

        # ---------------- attention + linear + mean ----------------
        with tc.tile_pool(name="attn", bufs=1) as ap_, \
             tc.tile_pool(name="psb", bufs=2) as ppb, \
             tc.tile_pool(name="pt_sb", bufs=1) as ptp, \
             tc.tile_pool(name="ybuf", bufs=1) as ybp, \
             tc.tile_pool(name="s_ps", bufs=6, space="PSUM") as sps, \
             tc.tile_pool(name="t_ps", bufs=2, space="PSUM") as tps, \
             tc.tile_pool(name="yz_ps", bufs=1, space="PSUM") as yzp, \
             tc.tile_pool(name="scratch", bufs=2) as scr:

            Lbc = ap_.tile([P, T], dt.float32, tag="lbc", name="lbc")
            Linv = ap_.tile([P, TC], dt.float32, tag="linv", name="linv")

            # Software-pipelined over i-chunks: PE is in-order, so chunk c's
            # P-transposes are emitted AFTER chunk c+1's score matmuls — the
            # softmax (DVE max -> ACT exp) of chunk c hides under them.
            state = {}

            def stage_scores(ic):
                # dd-outer: the stationary Q slice stays loaded in the PE
                # across the 4 j-blocks (LDWEIGHTS amortized 4x)
                Sb = [sps.tile([P, 512], dt.float32, tag="s", name="s")
                      for _ in range(NB)]
                mx4 = scr.tile([P, NB], dt.float32, tag="mx4", name="mx4")
                for dd in range(DC):
                    for jb in range(NB):
                        nc.tensor.matmul(Sb[jb][:],
                                         Q_dt[dd][:, ic * P:(ic + 1) * P],
                                         K_dt[dd][:, jb * 512:(jb + 1) * 512],
                                         start=(dd == 0), stop=(dd == DC - 1),
                                         skip_group_check=True)
                for jb in range(NB):
                    nc.vector.tensor_reduce(mx4[:, jb:jb + 1], Sb[jb][:],
                                            axis=AX.X, op=ALU.max)
                state[ic] = (Sb, mx4)

            def stage_softmax(ic):
                Sb, mx4 = state[ic]
                mx = scr.tile([P, 1], dt.float32, tag="mx", name="mx")
                nc.vector.tensor_reduce(mx[:], mx4[:], axis=AX.X, op=ALU.max)
                negb = scr.tile([P, 1], dt.float32, tag="negb", name="negb")
                nc.vector.tensor_scalar_mul(negb[:], mx[:], -float(SCALE))
                lp = scr.tile([P, NB], dt.float32, tag="lp", name="lp")
                Pex = ppb.tile([P, T], mdt, tag="pex", name="pex")
                for jb in range(NB):
                    nc.scalar.activation(Pex[:, jb * 512:(jb + 1) * 512],
                                         Sb[jb][:], AF.Exp,
                                         bias=negb[:], scale=float(SCALE),
                                         accum_out=lp[:, jb:jb + 1])
                state[ic] = (Pex, lp)

            def stage_transpose(ic, PT):
                Pex, lp = state.pop(ic)
                s_i = ic % 4
                for g in range(TC // 4):
                    tp = tps.tile([P, 4, P], mdt, tag="tp", name="tp")
                    for s in range(4):
                        jc = g * 4 + s
                        nc.tensor.transpose(tp[:, s, :],
                                            Pex[:, jc * P:(jc + 1) * P], identm[:])
                    copy_ps(PT[:, g * 4:(g + 1) * 4, s_i * P:(s_i + 1) * P], tp[:])
                # 1/l and its free-dim broadcast (needed only at block end)
                lsum = scr.tile([P, 1], dt.float32, tag="lsum", name="lsum")
                nc.vector.tensor_reduce(lsum[:], lp[:], axis=AX.X, op=ALU.add)
                nc.vector.reciprocal(Linv[:, ic:ic + 1], lsum[:])
                lt = tps.tile([P, 4, P], dt.float32, tag="tp", name="lt")
                nc.tensor.transpose(lt[:, 0, :],
                                    Linv[:, ic:ic + 1].to_broadcast([P, P]),
                                    ident[:])
                copy_ps(Lbc[:, ic * P:(ic + 1) * P], lt[:, 0, :])

            def stage_block_tail(bo, PT):
                # y_di[d, i] = (sum_j v_td[j, d] * PT[j, i]) / l_i
                Yb = ybp.tile([P, DC, 512], mdt, tag="yb", name="yb")
                for dd in range(DC):
                    yp = tps.tile([P, 512], dt.float32, tag="tp", name="yp")
                    for jc in range(TC):
                        nc.tensor.matmul(yp[:], V_td[jc][:, dd * P:(dd + 1) * P],
                                         PT[:, jc, :],
                                         start=(jc == 0), stop=(jc == TC - 1))
                    nc.vector.tensor_tensor(out=Yb[:, dd, :], in0=yp[:],
                                            in1=Lbc[:, bo * 512:(bo + 1) * 512],
                                            op=ALU.mult)
                # linear + relu + partial sum over t for this i-block
                for do in range(DC):
                    zp = tps.tile([P, 512], dt.float32, tag="tp", name="zp")
                    for dd in range(DC):
                        nc.tensor.matmul(zp[:], LwT[dd][:, do * P:(do + 1) * P],
                                         Yb[:, dd, :],
                                         start=(dd == 0), stop=(dd == DC - 1))
                    zr = scr.tile([P, 512], dt.float32, tag="zr", name="zr")
                    nc.scalar.activation(zr[:], zp[:], AF.Relu,
                                         bias=linb[:, do:do + 1], scale=1.0,
                                         accum_out=zsum[do][:, bo:bo + 1])

            PTs = {}
            stage_scores(0)
            for ic in range(T // P):
                bo = ic // 4
                if ic % 4 == 0:
                    PTs[bo] = ptp.tile([P, TC, 512], mdt, tag="pt", name="pt")
                stage_softmax(ic)
                if ic + 1 < T // P:
                    stage_scores(ic + 1)
                stage_transpose(ic, PTs[bo])
                if ic % 4 == 3:
                    stage_block_tail(bo, PTs.pop(bo))

            # ---------------- classifier ----------------
            ysum = [scr.tile([P, 1], dt.float32, tag=f"ys{d}", name=f"ys{d}") for d in range(DC)]
            for do in range(DC):
                nc.vector.tensor_reduce(ysum[do][:], zsum[do][:], axis=AX.X, op=ALU.add)
            op = tps.tile([P, 4, P], dt.float32, tag="tp", name="tp")
            for dd in range(DC):
                nc.tensor.matmul(op[:1, 0, :1], clfw[:, dd:dd + 1], ysum[dd][:],
                                 start=(dd == 0), stop=(dd == DC - 1))
            osb = scr.tile([1, 1], dt.float32, tag="osb", name="osb")
            nc.scalar.activation(osb[:], op[:1, 0, :1], AF.Sigmoid,
                                 bias=clfb[:], scale=float(1.0 / T))
            nc.sync.dma_start(out=out_ap, in_=osb[:])


def _get_nc(iters=1, mm_dtype=None):
    if mm_dtype is None:
        mm_dtype = os.environ.get("KERNEL_MM_DTYPE", "f32r")
    key = (iters, mm_dtype)
    if key not in _COMPILED:
        _COMPILED[key] = _build(iters=iters, mm_dtype=mm_dtype)
    return _COMPILED[key]


def _in_maps(x, embed, W_q, W_k, W_v, lin_w, lin_b, clf_w, clf_b):
    x = np.ascontiguousarray(np.asarray(x).astype(np.int32))
    common = {
        "embed": np.ascontiguousarray(np.asarray(embed, np.float32)),
        "W_q": np.ascontiguousarray(np.asarray(W_q, np.float32)),
        "W_k": np.ascontiguousarray(np.asarray(W_k, np.float32)),
        "W_v": np.ascontiguousarray(np.asarray(W_v, np.float32)),
        "lin_w": np.ascontiguousarray(np.asarray(lin_w, np.float32)),
        "lin_b": np.ascontiguousarray(np.asarray(lin_b, np.float32).reshape(D)),
        "clf_w": np.ascontiguousarray(np.asarray(clf_w, np.float32).reshape(D)),
        "clf_b": np.ascontiguousarray(np.asarray(clf_b, np.float32).reshape(1)),
    }
    return [dict(common, x=x[c]) for c in range(N_CORES)]


def kernel(x, embed, W_q, W_k, W_v, lin_w, lin_b, clf_w, clf_b):
    from concourse.bass_utils import run_bass_kernel_spmd

    nc = _get_nc()
    in_maps = _in_maps(x, embed, W_q, W_k, W_v, lin_w, lin_b, clf_w, clf_b)
    res = run_bass_kernel_spmd(nc, in_maps, core_ids=list(range(N_CORES)))
    out = np.stack([res.results[c]["out"][0, 0] for c in range(N_CORES)])
    return out.reshape(B, 1).astype(np.float32)

